# revision 7
# baseline (speedup 1.0000x reference)
"""CosineAttention Trainium2 Bass kernel (fp8 DoubleRow version).

Computes, per batch element b (one NeuronCore each, 8 cores total):
    proj   = x[b] @ W                      # [S, D]
    normed = proj / max(||proj||_2, eps)   # L2 normalize rows
    sim    = normed @ normed.T             # [S, S]
    out[b] = mean_s sigmoid(sim[s, t])     # [1, S]

Strategy (per core):
  - x[b] loaded as [128, D] fp32(-r) row tiles, transposed on the TensorE
    (fp32r identity matmul, 1.5 cyc/row) into PSUM, then cast fp32->fp8e4
    while copying into xT8 [128, 2, S] (DoubleRow pair layout: slot i holds
    d-chunk 2g+i), copies alternating between ACT and DVE.
  - proj is computed transposed via fp8 DoubleRow matmuls (lhsT = W8
    [128, 2, 128] pair slices, rhs = xT8, 4 accumulating matmuls per
    512-col block instead of 8 bf16).
  - Row norms: ACT copies PSUM->SBUF fp32 (psb), DVE squares to bf16, a
    ones-vector bf16 matmul reduces over partitions into norm2 [1, SB];
    rsqrt via ACT-sqrt + Newton step; rn scaled by 16 so normed8 = 16 *
    normed fits fp8e4 well; broadcast via K=1 ones matmul; DVE multiplies
    psb * bc -> normed8 [128, 2, S] fp8 (DoubleRow pair layout).
  - sim is symmetric: only tiles [i, j] with i < min(NST, 4j+4) are
    computed, each as 4 fp8 DoubleRow matmuls (256 x 16 = 1024 effective
    contraction). Sigmoid applied on ACT with scale=1/256 (undoing the
    16x16 normed scaling) + free-axis row-sum via accum_out; sigmoid
    output goes to fp8 pair tiles so the symmetry column sums are also
    DoubleRow ones-matmuls (one per tile pair).
  - out[t] = (rowsum + colsum) / S, assembled [128, 32], strided DMA out.
"""

from contextlib import ExitStack

import numpy as np

import concourse.bacc as bacc
import concourse.mybir as mybir
import concourse.tile as tile
from concourse.masks import make_identity

FP32 = mybir.dt.float32
FP32R = mybir.dt.float32r
BF16 = mybir.dt.bfloat16
FP8 = mybir.dt.float8e4
AF = mybir.ActivationFunctionType
ALU = mybir.AluOpType
AX = mybir.AxisListType
DR = mybir.MatmulPerfMode.DoubleRow

B = 8
S = 4096
D = 1024
EPS = 1e-12
N_CORES = 8
NSCALE = 16.0           # normed8 = NSCALE * normed (fp8 range use)


def emit(ctx, tc, out_ap, x_ap, w_ap, s_total=S, repeats=1):
    nc = tc.nc
    SB = 512                # s-block (matmul N)
    NB = s_total // SB      # number of s blocks
    NK = D // 128           # 128-row contraction chunks
    NG = NK // 2            # DoubleRow pair groups
    NST = s_total // 128    # 128-row s tiles
    STPB = SB // 128        # s tiles per block

    const_pool = ctx.enter_context(tc.tile_pool(name="const", bufs=1))
    wpool = ctx.enter_context(tc.tile_pool(name="wpool", bufs=1))
    normed_pool = ctx.enter_context(tc.tile_pool(name="normed", bufs=1))
    xraw_pool = ctx.enter_context(tc.tile_pool(name="xraw", bufs=12))
    xt_pool = ctx.enter_context(tc.tile_pool(name="xt", bufs=8))
    sq_pool = ctx.enter_context(tc.tile_pool(name="sq", bufs=3))
    projsb_pool = ctx.enter_context(tc.tile_pool(name="projsb", bufs=9))
    small_pool = ctx.enter_context(tc.tile_pool(name="small", bufs=8))
    attn_pool = ctx.enter_context(tc.tile_pool(name="attnsb", bufs=4))
    acc_pool = ctx.enter_context(tc.tile_pool(name="acc", bufs=1))

    identf = const_pool.tile([128, 128], FP32, name="identf")
    make_identity(nc, identf)
    ident = const_pool.tile([128, 128], FP32R, name="ident")
    nc.vector.tensor_copy(out=ident, in_=identf)
    ones_col = const_pool.tile([128, 1], BF16, name="ones_col")
    nc.vector.memset(ones_col, 1.0)
    ones8_t = const_pool.tile([128, 2, 16], FP8, name="ones8")
    nc.vector.memset(ones8_t, 1.0)
    ones8 = ones8_t[:, :, 0:1]
    ones_row_f = const_pool.tile([1, 128], FP32, name="ones_row_f")
    nc.vector.memset(ones_row_f, 1.0)
    ones_row = const_pool.tile([1, 128], FP32R, name="ones_row")
    nc.vector.tensor_copy(out=ones_row, in_=ones_row_f)
    out_all = const_pool.tile([128, NST], FP32, name="out_all")
    out_fin = const_pool.tile([128, NST], FP32, name="out_fin")
    sym_all = const_pool.tile([128, NST], FP32, name="sym_all")

    # prefetch x block 0 before W so PE transposes can start ASAP
    xr_next = []
    for st in range(STPB):
        xr = xraw_pool.tile([128, D], FP32, name="xr", tag="xr")
        nc.sync.dma_start(out=xr, in_=x_ap[st * 128:(st + 1) * 128, :])
        xr_next.append(xr)

    # W -> fp8 DoubleRow pair layout: w8[g][p, i, e] = W[128*(2g+i)+p, e]
    w8 = []
    for g in range(NG):
        w8.append(wpool.tile([128, 2, D], FP8, name=f"w8_{g}"))
    for c in range(NK):
        wf = xraw_pool.tile([128, D], FP32, name="wf", tag="xr")
        nc.sync.dma_start(out=wf, in_=w_ap[c * 128:(c + 1) * 128, :])
        nc.vector.tensor_copy(out=w8[c // 2][:, c % 2, :], in_=wf)

    normed8 = []
    for g in range(NG):
        normed8.append(
            normed_pool.tile([128, 2, s_total], FP8, name=f"normed8_{g}"))

    for _rep in range(repeats):
        nc.vector.memset(sym_all, 0.0)
        if _rep > 0:
            xr_next = []
            for st in range(STPB):
                xrt = xraw_pool.tile([128, D], FP32, name="xr", tag="xr")
                nc.sync.dma_start(out=xrt, in_=x_ap[st * 128:(st + 1) * 128, :])
                xr_next.append(xrt)

        # ---- Phase 1: xT8, proj (DoubleRow), norms, normed8 ----
        with ExitStack() as ph1:
            tr_ps = ph1.enter_context(
                tc.tile_pool(name="tr_ps", bufs=2, space="PSUM"))
            proj_ps = ph1.enter_context(
                tc.tile_pool(name="proj_ps", bufs=2, space="PSUM"))
            n2_ps = ph1.enter_context(
                tc.tile_pool(name="n2_ps", bufs=1, space="PSUM"))
            bc_ps = ph1.enter_context(
                tc.tile_pool(name="bc_ps", bufs=1, space="PSUM"))

            for j in range(NB):
                xr = xr_next
                if j + 1 < NB:
                    xr_next = []
                    for st in range(STPB):
                        xrt = xraw_pool.tile([128, D], FP32, name="xr",
                                             tag="xr")
                        s0 = (j + 1) * SB + st * 128
                        nc.sync.dma_start(out=xrt, in_=x_ap[s0:s0 + 128, :])
                        xr_next.append(xrt)

                # transpose + fp8 pair-pack: xt8[g][p, i, st*128+r] =
                #   x[j*SB + st*128 + r, 128*(2g+i)+p]
                xt8 = []
                for g in range(NG):
                    xt8.append(xt_pool.tile([128, 2, SB], FP8, name="xt8",
                                            tag="xt"))
                nco = 0
                for g in range(NG):
                    for sp in range(STPB // 2):
                        tr = tr_ps.tile([128, 4, 128], FP32, name="tr",
                                        tag="tp")
                        for q in range(4):
                            st = 2 * sp + q // 2
                            c = 2 * g + (q % 2)
                            nc.tensor.transpose(
                                tr[:, q, :],
                                xr[st][:, c * 128:(c + 1) * 128], identf)
                        for h in range(2):
                            st = 2 * sp + h
                            eng = nc.scalar if (nco % 2 == 0) else nc.vector
                            nco += 1
                            if eng is nc.scalar:
                                eng.copy(
                                    xt8[g][:, :, st * 128:(st + 1) * 128],
                                    tr[:, 2 * h:2 * h + 2, :])
                            else:
                                eng.tensor_copy(
                                    out=xt8[g][:, :, st * 128:(st + 1) * 128],
                                    in_=tr[:, 2 * h:2 * h + 2, :])

                n2 = n2_ps.tile([1, SB], FP32, name="n2", tag="n2")
                pend_n2 = []
                projs = []
                for e in range(NK):
                    pp = proj_ps.tile([128, SB], FP32, name="pp", tag="pp")
                    for g in range(NG):
                        nc.tensor.matmul(
                            pp,
                            lhsT=w8[g][:, :, e * 128:(e + 1) * 128],
                            rhs=xt8[g],
                            start=(g == 0),
                            stop=(g == NG - 1),
                            perf_mode=DR,
                        )
                    psb = projsb_pool.tile([128, SB], FP32, name="psb",
                                           tag="psb")
                    nc.scalar.copy(psb, pp)
                    sq = sq_pool.tile([128, SB], BF16, name="sq", tag="sq")
                    nc.vector.tensor_mul(sq, psb, psb)
                    projs.append(psb)
                    pend_n2.append((sq, e == 0, e == NK - 1))
                    if e >= 2:
                        sqd, st0, st1 = pend_n2.pop(0)
                        nc.tensor.matmul(n2, lhsT=ones_col, rhs=sqd,
                                         start=st0, stop=st1)
                for sqd, st0, st1 in pend_n2:
                    nc.tensor.matmul(n2, lhsT=ones_col, rhs=sqd,
                                     start=st0, stop=st1)
                pend_n2 = []

                # rn = NSCALE / max(sqrt(norm2), eps), Newton step on sqrt
                y = small_pool.tile([1, SB], FP32, name="y", tag="sm")
                nc.scalar.activation(out=y, in_=n2, func=AF.Sqrt)
                t1 = small_pool.tile([1, SB], FP32, name="t1", tag="sm")
                nc.vector.reciprocal(t1, y)
                h = small_pool.tile([1, SB], FP32, name="h", tag="sm")
                nc.vector.tensor_mul(h, n2, t1)
                nc.vector.tensor_add(h, h, y)
                nc.vector.tensor_scalar_mul(h, h, 0.5)
                nc.vector.tensor_scalar_max(h, h, EPS)
                rn = small_pool.tile([1, SB], FP32, name="rn", tag="sm")
                nc.vector.reciprocal(rn, h)
                nc.vector.tensor_scalar_mul(rn, rn, NSCALE)
                rn_r = small_pool.tile([1, SB], FP32R, name="rn_r", tag="sm")
                nc.vector.tensor_copy(out=rn_r, in_=rn)

                bc = bc_ps.tile([128, SB], FP32, name="bc", tag="bc")
                nc.tensor.matmul(bc, lhsT=ones_row, rhs=rn_r,
                                 start=True, stop=True)
                for e in range(NK):
                    nc.vector.tensor_mul(
                        normed8[e // 2][:, e % 2, j * SB:(j + 1) * SB],
                        projs[e], bc)

        # ---- Phase 2: sim (DoubleRow, upper super-block triangle) ----
        with ExitStack() as ph2:
            attn_ps = ph2.enter_context(
                tc.tile_pool(name="attn_ps", bufs=4, space="PSUM"))
            cs_ps = ph2.enter_context(
                tc.tile_pool(name="cs_ps", bufs=2, space="PSUM"))
            tps_ps = ph2.enter_context(
                tc.tile_pool(name="tps_ps", bufs=2, space="PSUM"))

            accs = [acc_pool.tile([128, NB], FP32, name=f"acc{i}")
                    for i in range(NST)]

            def flush_cs():
                while pend_cs:
                    csx, sp, st0, st1, _ = pend_cs.pop(0)
                    nc.tensor.matmul(csx, lhsT=ones8, rhs=sp,
                                     start=st0, stop=st1, perf_mode=DR)
                while pend_fin:
                    csx, jx = pend_fin.pop(0)
                    cs_sb = small_pool.tile([1, SB], FP32, name="cs_sb",
                                            tag="sm")
                    nc.vector.tensor_copy(out=cs_sb, in_=csx)
                    for c in range(STPB):
                        tp2 = tps_ps.tile([128, 1], FP32, name="tp2",
                                          tag="tp2")
                        nc.tensor.transpose(
                            tp2, cs_sb[:, c * 128:(c + 1) * 128],
                            identf[0:1, 0:1])
                        nc.vector.tensor_copy(
                            out=sym_all[:, STPB * jx + c: STPB * jx + c + 1],
                            in_=tp2)

            pend_cs = []
            pend_fin = []
            for j in range(NB):
                n_i = min(NST, 4 * j + 4)
                n_cs = min(NST, 4 * j)
                cs = None
                if n_cs > 0:
                    cs = cs_ps.tile([1, SB], FP32, name="cs", tag="cs")
                spair = None
                mid_flushed = False
                for i in range(n_i):
                    if i % 2 == 0:
                        spair = attn_pool.tile([128, 2, SB], FP8, name="scr",
                                               tag="scr")
                    apt = attn_ps.tile([128, SB], FP32, name="att", tag="att")
                    if i == 2 and not mid_flushed:
                        mid_flushed = True
                        flush_cs()
                    for g in range(NG):
                        nc.tensor.matmul(
                            apt,
                            lhsT=normed8[g][:, :, i * 128:(i + 1) * 128],
                            rhs=normed8[g][:, :, j * SB:(j + 1) * SB],
                            start=(g == 0),
                            stop=(g == NG - 1),
                            perf_mode=DR,
                        )
                    nc.scalar.activation(out=spair[:, i % 2, :], in_=apt,
                                         func=AF.Tanh,
                                         scale=0.5 / (NSCALE * NSCALE),
                                         accum_out=accs[i][:, j:j + 1])
                    if i % 2 == 1 and i < n_cs:
                        pend_cs.append((cs, spair, i == 1, i == n_cs - 1, j))
                if n_cs > 0:
                    pend_fin.append((cs, j))

            flush_cs()
            for i in range(NST):
                nc.vector.tensor_reduce(out_all[:, i:i + 1],
                                        accs[i][:, i // STPB:NB],
                                        axis=AX.X, op=ALU.add)
            nc.vector.tensor_add(out_fin, out_all, sym_all)
            nc.vector.tensor_scalar_mul(out_fin, out_fin, 0.5 / s_total)
            nc.vector.tensor_scalar_add(out_fin, out_fin, 0.5)
            nc.sync.dma_start(out=out_ap.rearrange("(i p) -> p i", p=128),
                              in_=out_fin)


def build(s_total=S, repeats=1):
    nc = bacc.Bacc("TRN2", target_bir_lowering=False, debug=False)
    x_t = nc.dram_tensor("x", [s_total, D], FP32, kind="ExternalInput")
    w_t = nc.dram_tensor("w", [D, D], FP32, kind="ExternalInput")
    o_t = nc.dram_tensor("out", [s_total], FP32, kind="ExternalOutput")
    with tile.TileContext(nc) as tc:
        with ExitStack() as ctx:
            emit(ctx, tc, o_t[:], x_t[:, :], w_t[:, :], s_total=s_total,
                 repeats=repeats)
    nc.compile()
    return nc


def _run(x, W, trace=False, **kwargs):
    from concourse.bass_utils import run_bass_kernel_spmd

    x = np.ascontiguousarray(np.asarray(x, dtype=np.float32))
    W = np.ascontiguousarray(np.asarray(W, dtype=np.float32))
    assert x.shape == (B, S, D) and W.shape == (D, D)
    nc = build()
    in_maps = [{"x": np.ascontiguousarray(x[b]), "w": W} for b in range(B)]
    res = run_bass_kernel_spmd(nc, in_maps, core_ids=list(range(N_CORES)),
                               trace=trace, **kwargs)
    out = np.stack([r["out"] for r in res.results])[:, None, :]
    return out.astype(np.float32), res


def kernel(x, W):
    out, _ = _run(x, W)
    return out
